# revision 6
# baseline (speedup 1.0000x reference)
"""Trainium2 Bass kernel for nn_CircularConvolution_5403068858821.

The reference computes result[:, :, n] += 1 for m in range(M) -> a constant
tensor of shape [N, C, L_x + M - 1] filled with M (=16.0). The inputs never
contribute arithmetically, so the kernel is a pure HBM fill: each of the 8
cores materializes its shard of the output on device and the host only
reassembles and dtype-converts.

Representation: every output element is exactly 16.0, which uint8 represents
exactly, so each core's shard is materialized as 2048*4111 = 8,419,328
uint8 bytes (value 16) and the host converts to float32 -- a 4x cut in HBM
write traffic vs f32.

Fill engine: gpsimd `kv_writeback` (attn ucode library). One instruction
writes, for each of batch*d_head (dhi,dho) slots, ncn contiguous bytes at a
column offset read from an SBUF ctx-idx tile; the hardware emits one 64KB
DMA descriptor per 16 slots. Viewing the shard as a flat byte buffer, a
single writeback with d_head=2048, ncn=4096 paints the first 8 MB (129
descriptors) and a second with d_head=128, ncn=240 paints the remaining
30,720 bytes (9 descriptors). The SBUF source is one [128, 1024] f32 tile
memset on DVE to 0x10101010 (four 0x10 bytes per f32 elem) and bitcast to
[128, 4096] uint8; the out AP's dho dim is declared stride-0 so batch_step=0
and the ucode re-reads the same tile for every dho slot. The ctx idxs reuse
the Bass const-AP float32-0.0 tile (bitcast: int32 zeros) with a completion
semaphore attached to its init memset.

Schedule and stream flattening: both writebacks are prepare_only with
their descriptor generation overlapping the DVE src memset and the main
DMA transfer; every semaphore wait is inlined into the consuming
instruction's on_wait list (no standalone wait instructions); Pool and DVE
body instructions are hoisted into the init block so neither engine pays a
block-entry branch; and the init/exit all-engine barriers, const-AP
memsets (except float32-0.0), Pool/DVE register moves and drains, and the
final waiter's postlude are dropped. The Pool chain (const memset 95ns ->
lib load 95ns -> desc gen 1038ns -> sem prop) and the DVE chain (memset
1127ns -> prop) converge within 3ns of each other at the first trigger.
All transformations validated in CoreSim (race detector on) and on HW.

Sharding: data-parallel over batch N=32 -> 4 batches/core.
"""

import os
import time

import numpy as np

import bass_rust as _bass_rust
import concourse.bass as bass
import concourse.mybir as mybir
from concourse import library_config
from concourse.bass_utils import run_bass_kernel_spmd

# Problem constants (hardcoded per the grading contract).
N, C, L_X = 32, 512, 4096
M = 16
L = L_X + M - 1  # 4111
N_CORES = 8
N_SHARD = N // N_CORES  # 4 batches per core
ROWS = N_SHARD * C  # 2048 rows per core

TOTAL = ROWS * L  # 8,419,328 uint8 per core
MAIN = 128 * 16 * 4096  # 8,388,608 bytes: d_head=2048, ncn=4096
TAIL_NCN = (TOTAL - MAIN) // 128  # 240: d_head=128, ncn=240
# f32 whose 4 bytes are each 0x10 (=16): memset 1024 f32 -> 4096 uint8 16s
F32_PATTERN = float(np.uint32(0x10101010).view(np.float32))

_CACHED_NC = None
LAST_RESULTS = None  # test harness introspection: last BassKernelResults


def _inject_wait(binst, sem, value):
    """Inline a semaphore wait into an instruction's on_wait list (saves a
    standalone EventSemaphore instruction's sequencer slot)."""
    ins = binst.ins
    w = mybir.SyncWait(sync_type="semaphore", id=sem.num, ant_name=sem.name,
                       wait_mode="sem-ge-imm", wait_value=value, wait_reg=None)
    si = ins.sync_info
    if si is None:
        ins.sync_info = mybir.SyncInfo(on_wait=[w], on_update=[])
    else:
        si.on_wait.append(w)


def _build_nc():
    """Emit the per-core Bass program: fill TOTAL uint8 with 16.

    Cost model (TimelineSim): 3730 ns -- ~1.33 us to first transfer,
    1468+12 ns of DMA transfers, ~0.93 us DMA-completion semaphore
    propagation + final wait.
    """
    nc = bass.Bass()
    out = nc.dram_tensor("out", [1, TOTAL], mybir.dt.uint8, kind="ExternalOutput")
    with (
        nc.Block() as block,
        nc.semaphore("psem") as psem,
        nc.semaphore("isem") as isem,
        nc.semaphore("dsem") as dsem,
        nc.sbuf_tensor("src", [128, 1024], mybir.dt.float32) as src_t,
    ):
        idx_ap = nc.const_aps.aps[(mybir.dt.float32, 0.0)].bitcast(mybir.dt.int32)
        src_u8 = src_t[:].bitcast(mybir.dt.uint8)
        out_main = out[0, :MAIN].rearrange("(b p d n) -> b p d n", b=1, p=128, d=16)
        out_tail = out[0, MAIN:].rearrange("(b p d n) -> b p d n", b=1, p=128, d=1)
        src_main = (
            src_u8[:, :4096]
            .rearrange("p (d b n) -> p d b n", d=1, b=1)
            .broadcast_to([128, 16, 1, 4096])
        )
        src_tail = src_u8[:, :TAIL_NCN].rearrange("p (d b n) -> p d b n", d=1, b=1)

        @block.vector
        def _(v):
            v.memset(src_t[:], F32_PATTERN).then_inc(psem, 1)

        @block.gpsimd
        def _(g):
            g.load_library(library_config.attn)
            kv1 = g.kv_writeback(out_main, src_main, idx_ap,
                                 prepare_only=True, sem=dsem)
            kv1.then_inc(psem, 1)
            _inject_wait(kv1, isem, 1)
            tr1 = g.trigger_dma(1)
            _inject_wait(tr1, psem, 2)  # main descs written + src memset done
            kv2 = g.kv_writeback(out_tail, src_tail, idx_ap,
                                 prepare_only=True, sem=dsem)
            kv2.then_inc(psem, 1)
            tr2 = g.trigger_dma(1)
            _inject_wait(tr2, psem, 3)

        @block.sync
        def _(s):
            s.wait_ge(dsem, 32)

        isem_h = isem

    # Post-process: strip idle-path instructions, flatten Pool/DVE streams
    # into the init block. Validated in CoreSim (race detector on) + HW.
    fn = nc.m.functions[0]
    hoist = {"Pool": [], "DVE": []}
    for bi, blk in enumerate(fn.blocks):
        keep = []
        for inst in blk.instructions:
            tn = type(inst).__name__
            nm = inst.name or ""
            eng = str(inst.engine).split(".")[-1] if hasattr(inst, "engine") else ""
            if nm.startswith("barrier_") and tn == "InstEventSemaphore":
                continue
            if bi == 0 and tn == "InstMemset":
                outs = getattr(inst, "outs", [])
                ref = "".join(str(getattr(o, "memref", "")) for o in outs)
                if "const-" in ref:
                    if "float32-0.0" not in ref:
                        continue
                    _bass_rust.then_inc(inst, isem_h, 1, False)
            if bi == 0 and tn == "InstRegisterMove" and eng in ("Pool", "DVE"):
                continue
            if bi == 0 and tn == "InstDrain" and eng in ("Pool", "DVE"):
                continue
            if eng in ("Pool", "DVE"):
                if tn in ("InstDrain", "InstUnconditionalBranch"):
                    continue
                if bi > 0:
                    hoist[eng].append(inst)
                    continue
            if bi > 0 and tn in ("InstDrain", "InstUnconditionalBranch") and eng == "SP":
                continue
            keep.append(inst)
        blk.instructions[:] = keep
    fn.blocks[0].instructions.extend(hoist["DVE"])
    fn.blocks[0].instructions.extend(hoist["Pool"])

    # Populate .instr bytes for the extended-inst InstISA subclasses
    # (LOAD_LIB etc.); without this walrus codegen fails "ISA wrong length".
    mybir.codegen_inst_isa_subclasses(nc)
    return nc


def kernel(x: np.ndarray, complex_weight: np.ndarray) -> np.ndarray:
    global _CACHED_NC, LAST_RESULTS
    if _CACHED_NC is None:
        _CACHED_NC = _build_nc()

    core_ids = list(range(N_CORES))
    in_maps = [{} for _ in core_ids]

    last_err = None
    for attempt in range(3):
        if attempt:
            time.sleep(60)  # axon terminal outages observed to self-recover
        try:
            res = run_bass_kernel_spmd(_CACHED_NC, in_maps, core_ids)
        except ModuleNotFoundError:
            # BASS_TRACE set but the axon NTFF profile hook isn't installed
            # in this container; retry with tracing hard-disabled.
            os.environ["BASS_NEVER_TRACE"] = "1"
            res = run_bass_kernel_spmd(_CACHED_NC, in_maps, core_ids)
        except Exception as e:  # transient tunnel/device failure
            last_err = e
            continue
        if all((np.asarray(res.results[c]["out"]) == M).all() for c in core_ids):
            break
        last_err = RuntimeError("device output failed full self-check")
    else:
        raise last_err
    LAST_RESULTS = res

    shards = [
        np.asarray(res.results[c]["out"])
        .reshape(N_SHARD, C, L)
        .astype(np.float32)
        for c in core_ids
    ]
    out = np.concatenate(shards, axis=0)
    return np.ascontiguousarray(out, dtype=np.float32)
